# revision 20
# baseline (speedup 1.0000x reference)
"""Bidirectional-ALiBi bias kernel for Trainium2 (Bass/Tile), 8-core SPMD.

Computes out[h, i, j] = |j - i| * m where m = alpha[h] on the first
row/column, gamma[h] above the diagonal, beta[h] below it, and 0 on the
(non-edge) diagonal.  Output [16, 2048, 2048] f32, sharded 2 heads/core.

Strategy ("aligned full-row tiles"): every interior row i is a shifted
window of a per-head profile V(k) = gamma*max(k,0) + beta*max(-k,0),
k = j - i.  Each core computes a per-head W image W[p, c] =
V(c - p - (S-1)) for c in [127, 4095), split into a hi chunk
[2047,4095) and lo chunk [127,2047).  Each 128-row output block t
becomes a PRIVATE SBUF tile [128, 2048] built from 1-2 window copies
out of W, then patched in SBUF: column 0 <- alpha*i (from R[p,t] =
alpha*(128t+p)) and, for t=0, row 0 <- alpha*j.  The patched tile goes
out as ONE fully contiguous, 8KB-per-row-aligned 1-MiB DMA.  No 4-byte
scatter writes, no misaligned row fragments (the old kernel lost ~25%
of DMA-engine time to those: 4B packets ran at 0.36 GB/s and k*512-byte
fragments at 14-23 GB/s vs 26.0 GB/s for aligned 8KB descriptors;
this version sustains 417-423 GB/s, the 16-engine wall, for the whole
~80us stream).

Engine layout (all measured): the DMA stream is the wall (16 SDMA
engines x 26.0 GB/s on the 32 MiB of writes).  Only sync (SP) and
scalar (Act) have HWDGE rings: sync carries the 16 h0 block DMAs,
scalar the 16 h1 ones.  Vector (DVE) computes the W chunks and feeds
the sync ring (h0 window copies + patches); the Act engine feeds its
own ring (h1 copies + patches via activation-copy) so each
copy->patch->DMA chain stays on one producer engine -- every extra
cross-engine hop costs ~1-2us of semaphore latency.  Gpsimd runs only
the 3 iotas: its tensor_scalar has a ~13us software launch cost that
must stay off the critical path (putting R/row-patch ops there cost
the first version 38us of ramp).  The t=0 tile's right half ships as a
separate DMA as soon as the copy+row-patch land, ~2us before the
column patch completes the left half.
"""

import numpy as np

H = 16
S = 2048
P = 128
N_CORES = 8
H_LOC = H // N_CORES  # 2 heads per core
NT = S // P  # 16 row blocks per head

C_LO = 127      # lowest W column needed (t=15 window starts at 2047-1920)
HI0 = 2047      # hi chunk covers c in [2047, 4095); lo covers [127, 2047)
W_HI = 2048
W_LO = HI0 - C_LO  # 1920

_NC = None


def _build(nbuf=10, copy_h1_eng="scalar", ring_h1="scalar"):
    import concourse.bacc as bacc
    import concourse.mybir as mybir
    from concourse.tile import TileContext

    f32 = mybir.dt.float32
    nc = bacc.Bacc("TRN2", target_bir_lowering=False, debug=False)

    alpha_d = nc.dram_tensor("alpha", [H_LOC], f32, kind="ExternalInput").ap()
    beta_d = nc.dram_tensor("beta", [H_LOC], f32, kind="ExternalInput").ap()
    gamma_d = nc.dram_tensor("gamma", [H_LOC], f32, kind="ExternalInput").ap()
    out_d = nc.dram_tensor("out", [H_LOC, S, S], f32, kind="ExternalOutput").ap()

    with TileContext(nc) as tc:
        rings = {"h0": nc.sync, "h1": getattr(nc, ring_h1)}
        copy_eng = {0: nc.vector, 1: getattr(nc, copy_h1_eng)}

        with (
            tc.tile_pool(name="coef", bufs=1) as cpool,
            tc.tile_pool(name="kpool", bufs=1) as kpool,
            tc.tile_pool(name="wpool", bufs=1) as wpool,
            tc.tile_pool(name="t2pool", bufs=2) as t2pool,
            tc.tile_pool(name="tpool", bufs=nbuf) as tpool,
        ):
            # per-head coefficients broadcast to all partitions: [128, 2].
            # G2/B2 gate the W compute; A2 only gates the (later) patches.
            G2 = cpool.tile([P, H_LOC], f32)
            nc.sync.dma_start(out=G2[:], in_=gamma_d.partition_broadcast(P))
            B2 = cpool.tile([P, H_LOC], f32)
            rings["h1"].dma_start(out=B2[:], in_=beta_d.partition_broadcast(P))
            A2 = cpool.tile([P, H_LOC], f32)
            nc.sync.dma_start(out=A2[:], in_=alpha_d.partition_broadcast(P))
            # NB2 emitted FIRST on vector: the greedy Tile scheduler picks
            # the highest-priority READY op whenever an engine frees up, so
            # every link of the first tile's chain must be dep-complete the
            # moment its predecessor retires or unrelated work cuts in.
            # With NB2 done up front, Whi0 is ready the instant T2hi0 ends.
            NB2 = cpool.tile([P, H_LOC], f32)
            nc.vector.tensor_scalar_mul(NB2[:], B2[:], -1.0)

            # K iotas: Khi[p, cc] = cc - p  (c = cc + 2047, k = c - p - 2047)
            #          Klo[p, cc] = cc - p - 1920  (c = cc + 127)
            Khi = kpool.tile([P, W_HI], f32, tag="Khi")
            nc.gpsimd.iota(
                Khi[:],
                pattern=[[1, W_HI]],
                base=0,
                channel_multiplier=-1,
                allow_small_or_imprecise_dtypes=True,
            )
            # IB[p, t] = 128t + p, for the column-0 patch sources
            IB = cpool.tile([P, NT], f32, tag="IB")
            nc.gpsimd.iota(
                IB[:],
                pattern=[[P, NT]],
                base=0,
                channel_multiplier=1,
                allow_small_or_imprecise_dtypes=True,
            )
            Klo = kpool.tile([P, W_LO], f32, tag="Klo")
            nc.gpsimd.iota(
                Klo[:],
                pattern=[[1, W_LO]],
                base=-W_LO,
                channel_multiplier=-1,
                allow_small_or_imprecise_dtypes=True,
            )
            Rs = [None, None]

            # W images.  T2 = max(gamma*k, 0); W = max(-beta*k, T2) -- the
            # two branches are never simultaneously positive so max = sum.
            # h1's T2 runs as Relu(k*gamma) on the Act engine (idle early,
            # in parallel with vector's h0 chain); the STT stays on vector.
            def w_chunk(Kc, w, h, Wout):
                T2 = t2pool.tile([P, W_HI], f32, tag="T2")
                if h == 1:
                    nc.scalar.activation(
                        out=T2[:, :w],
                        in_=Kc[:, :w],
                        func=mybir.ActivationFunctionType.Relu,
                        scale=G2[:, h : h + 1],
                    )
                else:
                    nc.vector.tensor_scalar(
                        out=T2[:, :w],
                        in0=Kc[:, :w],
                        scalar1=G2[:, h : h + 1],
                        scalar2=0.0,
                        op0=mybir.AluOpType.mult,
                        op1=mybir.AluOpType.max,
                    )
                nc.vector.scalar_tensor_tensor(
                    out=Wout[:],
                    in0=Kc[:, :w],
                    scalar=NB2[:, h : h + 1],
                    in1=T2[:, :w],
                    op0=mybir.AluOpType.mult,
                    op1=mybir.AluOpType.max,
                )

            Whi = [wpool.tile([P, W_HI], f32, tag=f"Whi{h}", name=f"Whi{h}") for h in range(H_LOC)]
            Wlo = [wpool.tile([P, W_LO], f32, tag=f"Wlo{h}", name=f"Wlo{h}") for h in range(H_LOC)]

            def cpy(h, out, in_):
                # h0 tiles copy on vector, h1 tiles on the Act engine, so
                # each DMA ring is fed by exactly one compute engine and the
                # copy -> patch -> dma chain never ping-pongs across engines
                # (each cross-engine semaphore hop costs ~1-2us).
                if copy_eng[h] is nc.vector:
                    nc.vector.tensor_copy(out=out, in_=in_)
                else:
                    copy_eng[h].copy(out=out, in_=in_)

            def emit_tile(h, t):
                if Rs[h] is None:
                    # R[h][p, t] = alpha_h * (128t + p): column-0 patch values
                    Rh = cpool.tile([P, NT], f32, tag=f"R{h}", name=f"R{h}")
                    nc.vector.tensor_scalar_mul(Rh[:], IB[:], A2[:, h : h + 1])
                    Rs[h] = Rh
                T = tpool.tile([P, S], f32, tag="T")
                ring = rings[f"h{h}"]
                half = S // 2
                # window c in [o, o+2048), o = 2047 - 128t
                if t == 0:
                    cpy(h, T[:], Whi[h][:])
                    # row-0 patch first (cheap, needs only Khi+A2):
                    # T[0, j] = alpha_h * j; Khi[0, cc] = cc = j
                    if h == 0:
                        nc.vector.tensor_scalar_mul(
                            T[0:1, :], Khi[0:1, :], A2[0:1, h : h + 1]
                        )
                    else:
                        nc.scalar.mul(T[0:1, :], Khi[0:1, :], A2[0:1, h : h + 1])
                    # the right half is now fully patched: ship it early
                    ring.dma_start(out=out_d[h, 0:P, half:S], in_=T[:, half:S])
                    # column-0 patch only gates the left half
                    cpy(h, T[:, 0:1], Rs[h][:, t : t + 1])
                    ring.dma_start(out=out_d[h, 0:P, 0:half], in_=T[:, 0:half])
                else:
                    jl = P * t  # low piece covers j in [0, 128t)
                    cpy(h, T[:, 0:jl], Wlo[h][:, W_LO - jl : W_LO])
                    cpy(h, T[:, jl:S], Whi[h][:, 0 : S - jl])
                    # column-0 patch: T[p, 0] = alpha_h * (128t + p)
                    cpy(h, T[:, 0:1], Rs[h][:, t : t + 1])
                    ring.dma_start(
                        out=out_d[h, P * t : P * (t + 1), :], in_=T[:]
                    )

            # hi chunks first (they alone serve t=0); interleave the lo
            # chunks between early tile emissions so the DMA stream never
            # starves while vector computes them.
            w_chunk(Khi, W_HI, 0, Whi[0])
            emit_tile(0, 0)
            w_chunk(Khi, W_HI, 1, Whi[1])
            emit_tile(1, 0)
            w_chunk(Klo, W_LO, 0, Wlo[0])
            emit_tile(0, 1)
            w_chunk(Klo, W_LO, 1, Wlo[1])
            emit_tile(1, 1)
            for t in range(2, NT):
                for h in range(H_LOC):
                    emit_tile(h, t)

    nc.compile()
    return nc


def _run(alpha, beta, gamma, **spmd_kwargs):
    """Compile (cached) and run on the 8 NeuronCores; returns BassKernelResults."""
    global _NC
    if _NC is None:
        _NC = _build()
    from concourse import bass_utils

    alpha = np.ascontiguousarray(alpha, dtype=np.float32)
    beta = np.ascontiguousarray(beta, dtype=np.float32)
    gamma = np.ascontiguousarray(gamma, dtype=np.float32)
    in_maps = [
        {
            "alpha": alpha[c * H_LOC : (c + 1) * H_LOC],
            "beta": beta[c * H_LOC : (c + 1) * H_LOC],
            "gamma": gamma[c * H_LOC : (c + 1) * H_LOC],
        }
        for c in range(N_CORES)
    ]
    return bass_utils.run_bass_kernel_spmd(
        _NC, in_maps, core_ids=list(range(N_CORES)), **spmd_kwargs
    )


def kernel(alpha, beta, gamma, seq_len):
    assert int(seq_len) == S, f"kernel hardcodes seq_len={S}, got {seq_len}"
    res = _run(alpha, beta, gamma)
    return np.concatenate([r["out"] for r in res.results], axis=0)


# revision 24
# speedup vs baseline: 1.1702x; 1.1702x over previous
"""Bidirectional-ALiBi bias kernel for Trainium2 (Bass/Tile), 8-core SPMD.

Computes out[h, i, j] = |j - i| * m where m = alpha[h] on the first
row/column, gamma[h] above the diagonal, beta[h] below it, and 0 on the
(non-edge) diagonal.  Output [16, 2048, 2048] f32, sharded 2 heads/core.

Strategy ("aligned full-row tiles"): every interior row i is a shifted
window of a per-head profile V(k) = gamma*max(k,0) + beta*max(-k,0),
k = j - i.  Each core computes a per-head W image W[p, c] =
V(c - p - (S-1)) for c in [127, 4095) in three chunks: hi_a
[3071,4095), hi_b [2047,3071), lo [127,2047) -- separate SBUF tiles
because Tile dependency tracking is tile-granular.  Each 128-row
output block t becomes a PRIVATE SBUF tile [128, 2048] built from 2-3
window copies out of the chunks, patched in SBUF (column 0 <- alpha*i
from R[p,t] = alpha*(128t+p); for t=0 also row 0 <- alpha*j), and
shipped as ONE fully contiguous, 8KB-per-row-aligned 1-MiB DMA (no
4-byte scatters, no misaligned fragments -- those cost the original
kernel ~25% of DMA-engine time; this version sustains 417-423 GB/s,
the 16-engine wall, for the whole ~80us stream).

The ramp is the only soft cost: coefficient broadcasts become usable
~14us in (DMA completion-semaphore latency, measured irreducible), so
block 0 is split: its right half (hi_a window, row-0-patched) ships as
a 0.5-MiB DMA ~3us before the left half's chain completes.  Engine
layout: vector computes W chunks + h0 copies/patches feeding the sync
ring; the Act engine computes h1's T2 = Relu(k*gamma) and h1
copies/patches feeding its own ring (chains never ping-pong across
engines -- each cross-engine semaphore hop costs ~1-2us); gpsimd runs
only the iotas (its tensor_scalar has a ~13us software launch cost).
NB2 is emitted first on vector so every link of block 0's chain is
dep-complete the moment its predecessor retires (the Tile scheduler
greedily inserts any READY op when an engine frees up).
"""

import numpy as np

H = 16
S = 2048
P = 128
N_CORES = 8
H_LOC = H // N_CORES  # 2 heads per core
NT = S // P  # 16 row blocks per head

HA = 1024        # hi_a: cc in [1024, 2048)  (c = cc + 2047)
HB = 1024        # hi_b: cc in [0, 1024)
W_LO = 1920      # lo:   c in [127, 2047), local u = c - 127

_NC = None


def _build(nbuf=10):
    import concourse.bacc as bacc
    import concourse.mybir as mybir
    from concourse.tile import TileContext

    f32 = mybir.dt.float32
    nc = bacc.Bacc("TRN2", target_bir_lowering=False, debug=False)

    alpha_d = nc.dram_tensor("alpha", [H_LOC], f32, kind="ExternalInput").ap()
    beta_d = nc.dram_tensor("beta", [H_LOC], f32, kind="ExternalInput").ap()
    gamma_d = nc.dram_tensor("gamma", [H_LOC], f32, kind="ExternalInput").ap()
    out_d = nc.dram_tensor("out", [H_LOC, S, S], f32, kind="ExternalOutput").ap()

    with TileContext(nc) as tc:
        rings = [nc.sync, nc.scalar]

        with (
            tc.tile_pool(name="coef", bufs=1) as cpool,
            tc.tile_pool(name="kpool", bufs=1) as kpool,
            tc.tile_pool(name="wpool", bufs=1) as wpool,
            tc.tile_pool(name="t2pool", bufs=2) as t2pool,
            tc.tile_pool(name="tpool", bufs=nbuf) as tpool,
        ):
            # per-head coefficients broadcast to all partitions: [128, 2]
            G2 = cpool.tile([P, H_LOC], f32)
            nc.sync.dma_start(out=G2[:], in_=gamma_d.partition_broadcast(P))
            B2 = cpool.tile([P, H_LOC], f32)
            nc.scalar.dma_start(out=B2[:], in_=beta_d.partition_broadcast(P))
            A2 = cpool.tile([P, H_LOC], f32)
            nc.sync.dma_start(out=A2[:], in_=alpha_d.partition_broadcast(P))
            NB2 = cpool.tile([P, H_LOC], f32)
            nc.vector.tensor_scalar_mul(NB2[:], B2[:], -1.0)

            def k_iota(name, width, base):
                Kt = kpool.tile([P, width], f32, tag=name)
                nc.gpsimd.iota(
                    Kt[:],
                    pattern=[[1, width]],
                    base=base,
                    channel_multiplier=-1,
                    allow_small_or_imprecise_dtypes=True,
                )
                return Kt

            # K[p, u] = (u + base) - p; tile t reads c = j + 2047 - 128t
            Ka = k_iota("Ka", HA, 1024)   # cc in [1024, 2048): val = j - i at t=0 right half
            # IB[p, t] = 128t + p, for the column-0 patch sources
            IB = cpool.tile([P, NT], f32, tag="IB")
            nc.gpsimd.iota(
                IB[:],
                pattern=[[P, NT]],
                base=0,
                channel_multiplier=1,
                allow_small_or_imprecise_dtypes=True,
            )
            Kb = k_iota("Kb", HB, 0)      # cc in [0, 1024)
            Klo = k_iota("Klo", W_LO, -W_LO)  # c in [127, 2047)
            Rs = [None, None]

            # T2 = max(gamma*k, 0); W = max(-beta*k, T2) (branches never
            # both positive).  h1's T2 runs as Relu(k*gamma) on the Act
            # engine, overlapping vector's h0 chain.
            def w_chunk(Kc, w, h, Wout):
                T2 = t2pool.tile([P, W_LO], f32, tag="T2")
                if h == 1:
                    nc.scalar.activation(
                        out=T2[:, :w],
                        in_=Kc[:, :w],
                        func=mybir.ActivationFunctionType.Relu,
                        scale=G2[:, h : h + 1],
                    )
                else:
                    nc.vector.tensor_scalar(
                        out=T2[:, :w],
                        in0=Kc[:, :w],
                        scalar1=G2[:, h : h + 1],
                        scalar2=0.0,
                        op0=mybir.AluOpType.mult,
                        op1=mybir.AluOpType.max,
                    )
                nc.vector.scalar_tensor_tensor(
                    out=Wout[:],
                    in0=Kc[:, :w],
                    scalar=NB2[:, h : h + 1],
                    in1=T2[:, :w],
                    op0=mybir.AluOpType.mult,
                    op1=mybir.AluOpType.max,
                )

            Wa = [wpool.tile([P, HA], f32, tag=f"Wa{h}", name=f"Wa{h}") for h in range(H_LOC)]
            Wb = [wpool.tile([P, HB], f32, tag=f"Wb{h}", name=f"Wb{h}") for h in range(H_LOC)]
            Wlo = [wpool.tile([P, W_LO], f32, tag=f"Wlo{h}", name=f"Wlo{h}") for h in range(H_LOC)]

            def cpy(h, out, in_):
                if h == 0:
                    nc.vector.tensor_copy(out=out, in_=in_)
                else:
                    nc.scalar.copy(out=out, in_=in_)

            def rowpatch(h, dst, Ksrc):
                # dst = alpha_h * j, with Ksrc's row 0 holding j
                if h == 0:
                    nc.vector.tensor_scalar_mul(dst, Ksrc, A2[0:1, h : h + 1])
                else:
                    nc.scalar.mul(dst, Ksrc, A2[0:1, h : h + 1])

            def mk_r(h):
                if Rs[h] is None:
                    # R[h][p, t] = alpha_h * (128t + p): column-0 patch values
                    Rh = cpool.tile([P, NT], f32, tag=f"R{h}", name=f"R{h}")
                    nc.vector.tensor_scalar_mul(Rh[:], IB[:], A2[:, h : h + 1])
                    Rs[h] = Rh

            T0s = [None, None]

            def emit_t0_right(h):
                # right half = Wa window exactly; ship it the moment the
                # copy + row patch land (the column patch only touches
                # the left half).  Ka[0, u] = 1024 + u = j on row 0.
                T = tpool.tile([P, S], f32, tag="T")
                T0s[h] = T
                cpy(h, T[:, HB:S], Wa[h][:])
                rowpatch(h, T[0:1, HB:S], Ka[0:1, :])
                rings[h].dma_start(out=out_d[h, 0:P, HB:S], in_=T[:, HB:S])

            def emit_t0_left(h):
                mk_r(h)
                T = T0s[h]
                cpy(h, T[:, 0:HB], Wb[h][:])
                rowpatch(h, T[0:1, 0:HB], Kb[0:1, :])
                cpy(h, T[:, 0:1], Rs[h][:, 0:1])
                rings[h].dma_start(out=out_d[h, 0:P, 0:HB], in_=T[:, 0:HB])

            def emit_tile(h, t):
                mk_r(h)
                T = tpool.tile([P, S], f32, tag="T")
                jl = P * t  # low piece covers j in [0, 128t)
                cpy(h, T[:, 0:jl], Wlo[h][:, W_LO - jl : W_LO])
                # hi cc range [0, 2048-128t): b part, then a part (t <= 7)
                wb = min(HB, S - jl)
                cpy(h, T[:, jl : jl + wb], Wb[h][:, 0:wb])
                if jl + wb < S:
                    cpy(h, T[:, jl + wb : S], Wa[h][:, 0 : S - jl - wb])
                cpy(h, T[:, 0:1], Rs[h][:, t : t + 1])
                rings[h].dma_start(out=out_d[h, P * t : P * (t + 1), :], in_=T[:])

            # hi_a chunk -> block-0 right half per head (earliest possible
            # first DMA on each ring), then hi_b -> left halves, then lo
            # chunks interleaved with the first full tiles.
            w_chunk(Ka, HA, 0, Wa[0])
            emit_t0_right(0)
            w_chunk(Ka, HA, 1, Wa[1])
            emit_t0_right(1)
            w_chunk(Kb, HB, 0, Wb[0])
            emit_t0_left(0)
            w_chunk(Kb, HB, 1, Wb[1])
            emit_t0_left(1)
            w_chunk(Klo, W_LO, 0, Wlo[0])
            emit_tile(0, 1)
            w_chunk(Klo, W_LO, 1, Wlo[1])
            emit_tile(1, 1)
            for t in range(2, NT):
                for h in range(H_LOC):
                    emit_tile(h, t)

    nc.compile()
    return nc


def _run(alpha, beta, gamma, **spmd_kwargs):
    """Compile (cached) and run on the 8 NeuronCores; returns BassKernelResults."""
    global _NC
    if _NC is None:
        _NC = _build()
    from concourse import bass_utils

    alpha = np.ascontiguousarray(alpha, dtype=np.float32)
    beta = np.ascontiguousarray(beta, dtype=np.float32)
    gamma = np.ascontiguousarray(gamma, dtype=np.float32)
    in_maps = [
        {
            "alpha": alpha[c * H_LOC : (c + 1) * H_LOC],
            "beta": beta[c * H_LOC : (c + 1) * H_LOC],
            "gamma": gamma[c * H_LOC : (c + 1) * H_LOC],
        }
        for c in range(N_CORES)
    ]
    return bass_utils.run_bass_kernel_spmd(
        _NC, in_maps, core_ids=list(range(N_CORES)), **spmd_kwargs
    )


def kernel(alpha, beta, gamma, seq_len):
    assert int(seq_len) == S, f"kernel hardcodes seq_len={S}, got {seq_len}"
    res = _run(alpha, beta, gamma)
    return np.concatenate([r["out"] for r in res.results], axis=0)
